# revision 16
# baseline (speedup 1.0000x reference)
"""Trainium2 Bass kernel for nn_AllAmplitude (helicity-amplitude intensity).

Math: the reference contracts two spin-1 Wigner-D matrices per (resonance,
event) with a Breit-Wigner weight and sums |amp|^2 over external helicities.
Because D1 @ D2 = D^1(U1 U2) for the SU(2) elements U1, U2 of the two
rotations, and sum_{a,dlt} mult_dlt M_r conj(M_r') = 2 tr D(V) + D(V)[0,0]
with V = U_r'^dag U_r, the whole intensity collapses to

  I = 7 sum_r |w_r|^2
    + sum_{r<r'} 2 Re(w_r conj(w_r')) (10 Re(av)^2 + 2 Im(av)^2 - 3)

with av = conj(a_r') a_r + b_r' conj(b_r), (a, b) the Cayley-Klein parameters
of the composed rotation, and w_r the complex Breit-Wigner weight.  Per event
this is ~200 flops instead of the reference's ~3000.

Sharding: pure data parallelism. The event axis N=262144 splits across the 8
NeuronCores (32768 events each, laid out [128 partitions x 256 events] with
the R=4 resonance slices side by side in the free axis).

Implementation (raw bass Block, manual semaphores; no Tile scheduler):
 - sin/cos of the four composite half-angles via fractional-turn range
   reduction (round-to-nearest through the 1.5*2^23 float trick), one fused
   custom-DVE op per angle/offset, feeding the ScalarE Sin spline;
 - chi and |w|^2 via a fused  a^2*s0 + b^2*s1 + s2  custom-DVE op;
 - bounded amplitude algebra in fp16 (2x DVE rate), Breit-Wigner in fp32,
   |w| pair products in bf16 (range);
 - ScalarE carries all transcendentals plus the 1-input affine ops;
 - per-resonance DMA-slice semaphores let the DVE start while inputs land.
"""

import numpy as np
from contextlib import ExitStack

import concourse.bass as bass
import concourse.tile as tile
from concourse import bacc, mybir
from concourse.bass_utils import run_bass_kernel_spmd

F32 = mybir.dt.float32
BF16 = mybir.dt.bfloat16
FP16 = mybir.dt.float16
ALU = mybir.AluOpType
ACTF = mybir.ActivationFunctionType

R = 4
N_TOTAL = 262144
N_CORES = 8
N_CORE = N_TOTAL // N_CORES     # 32768 events per core
P = 128                         # SBUF partitions
E = N_CORE // P                 # 256 events per partition per resonance
W = R * E                       # 1024 free-dim of a full working tile

MAGIC = float(np.float32(1.5 * 2.0**23))   # round-to-nearest-int bias trick
INV4PI = float(np.float32(1.0 / (4.0 * np.pi)))
TWOPI = float(np.float32(2.0 * np.pi))
HALFPI = float(np.float32(np.pi / 2.0))


INPUT_NAMES = ("alpha1", "beta1", "gamma1", "alpha2", "beta2", "gamma2", "m")


# ---------------------------------------------------------------------------
# custom fused DVE ops
# ---------------------------------------------------------------------------

def _register_custom_ops():
    import concourse.dve_ops as dve_ops
    from concourse.dve_spec import Spec, Src0, Src1, C0, C1, C2, sq, lower, _has_src1
    from concourse.dve_uop import DveOpSpec
    from concourse.dve_ops import DveOp

    if any(op.name == "ANT_RANGE_RED_ADD" for op in dve_ops.OPS):
        return {op.name: op for op in dve_ops.OPS}

    def make_op(name, spec):
        shas = {}
        for ver in ("v3", "v4"):
            uops = lower(spec, ver=ver)
            shas[ver] = DveOpSpec(name=name, opcode=31, uops=uops,
                                  rd1_en=_has_src1(spec)).sha(ver)
        return DveOp(name, spec, subdim=False, uops_sha=shas)

    def _rr_ref(sgn):
        def ref(in0, in1, s0, s1, imm2):
            t = ((in0 + sgn * in1) * s0 + s1).astype(np.float32)
            r = ((t + imm2).astype(np.float32) - imm2).astype(np.float32)
            return (t - r).astype(np.float32)
        return ref

    u = (Src0 + Src1) * C0 + C1
    rr_add = make_op("ANT_RANGE_RED_ADD",
                     Spec(body=u - ((u + C2) - C2), reference=_rr_ref(1.0)))
    u2 = (Src0 - Src1) * C0 + C1
    rr_sub = make_op("ANT_RANGE_RED_SUB",
                     Spec(body=u2 - ((u2 + C2) - C2), reference=_rr_ref(-1.0)))
    chi = make_op("ANT_CHI", Spec(
        body=sq(Src0) * C0 + sq(Src1) * C1 + C2,
        reference=lambda in0, in1, s0, s1, imm2:
            (in0 * in0 * s0 + in1 * in1 * s1 + imm2).astype(np.float32)))
    den = make_op("ANT_DEN", Spec(
        body=sq(C0 - Src0) + C1,
        reference=lambda in0, in1, s0, s1, imm2:
            ((s0 - in0) * (s0 - in0) + s1).astype(np.float32)))

    for op in (rr_add, rr_sub, chi, den):
        dve_ops.OPS.append(op)
        dve_ops._SUB_OPCODE_FOR_NAME[op.name] = (
            dve_ops._CUSTOM_DVE_ROW_BASE + len(dve_ops.OPS) - 1)
        dve_ops.CUSTOM_DVE_SPECS[op.name] = op.spec
    assert max(dve_ops._SUB_OPCODE_FOR_NAME.values()) < 0x20
    return {op.name: op for op in dve_ops.OPS}


def _rs(r):
    return slice(r * E, (r + 1) * E)


def build(m0, g0, coef_r, coef_i):
    OPS = _register_custom_ops()
    RR_ADD, RR_SUB, CHI, DEN = (OPS["ANT_RANGE_RED_ADD"], OPS["ANT_RANGE_RED_SUB"],
                                OPS["ANT_CHI"], OPS["ANT_DEN"])
    AT = FP16

    nc = bacc.Bacc("TRN2", target_bir_lowering=False, debug=False,
                   num_devices=N_CORES)
    ins = {k: nc.dram_tensor(k, (R, N_CORE), F32, kind="ExternalInput").ap()
           for k in INPUT_NAMES}
    out_ap = nc.dram_tensor("out", (N_CORE,), F32, kind="ExternalOutput").ap()

    f32 = np.float32
    m0 = m0.astype(np.float64); g0 = g0.astype(np.float64)
    cR = [float(f32(coef_r[r] * np.cos(coef_i[r]))) for r in range(R)]
    cI = [float(f32(coef_r[r] * np.sin(coef_i[r]))) for r in range(R)]
    m0sq = [float(f32(m0[r] * m0[r])) for r in range(R)]
    y = [float(f32(m0[r] * g0[r])) for r in range(R)]
    ysq = [float(f32(f32(y[r]) * f32(y[r]))) for r in range(R)]
    k1 = [float(f32(-f32(cI[r]) * f32(y[r]))) for r in range(R)]
    k2 = [float(f32(f32(cR[r]) * f32(y[r]))) for r in range(R)]

    # ---- static SBUF allocation ----
    alloc = []
    def sb(name, shape, dt=F32):
        t = nc.alloc_sbuf_tensor(name, list(shape), dt)
        alloc.append(t)
        return t.ap()

    tin = {k: sb(f"in_{k}", [P, W]) for k in INPUT_NAMES}
    pi2 = sb("pi2", [P, 1])
    ub = sb("ub", [P, W]); vb = sb("vb", [P, W])
    wb = sb("wb", [P, W]); zb = sb("zb", [P, W])
    fts = {n: sb(f"f_{n}", [P, W]) for n in
           ("As", "Ac", "Bs", "Bc", "Cs", "Cc", "Ds", "Dc")}
    # packed trig tiles: CB1=[cb1|sb1], CB2a=[cb2|sb2], CB2b=[sb2|cb2]
    CB1 = sb("CB1", [P, 2 * W], AT)
    CB2a = sb("CB2a", [P, 2 * W], AT)
    CB2b = sb("CB2b", [P, 2 * W], AT)
    SC1s = sb("SC1s", [P, 2 * W], AT)   # [snA|snB]
    SC1c = sb("SC1c", [P, 2 * W], AT)   # [csA|csB]
    SC2s = sb("SC2s", [P, 2 * W], AT)   # [snC|snD]
    SC2c = sb("SC2c", [P, 2 * W], AT)   # [csC|csD]
    MP12 = sb("MP12", [P, 2 * W], AT)   # [M1|M2]
    MP34 = sb("MP34", [P, 2 * W], AT)   # [M3|M4]
    PQT1 = sb("PQT1", [P, 4 * W], AT)   # [Ac|As|Bc|Bs]
    PQT2 = sb("PQT2", [P, 4 * W], AT)   # [Cc|Cs|Dc|Ds]
    AB4 = sb("AB4", [P, 4 * W], AT)   # [are|aim|bre|bim]
    are = AB4[:, 0:W]; aim = AB4[:, W:2*W]
    bre = AB4[:, 2*W:3*W]; bim = AB4[:, 3*W:4*W]
    msq = sb("msq", [P, W]); den = sb("den", [P, W]); rcp = sb("rcp", [P, W])
    wp1 = sb("wp1", [P, W]); wp2 = sb("wp2", [P, W])
    wre = sb("wre", [P, W]); wim = sb("wim", [P, W])
    WH = sb("WH", [P, 2 * W], BF16)   # [wreh|wimh]
    wreh = WH[:, 0:W]; wimh = WH[:, W:2*W]
    dall = sb("dall", [P, W]); dh = sb("dh", [P, 2 * E]); dg = sb("dg", [P, E])
    acc = sb("acc", [P, E])
    # pair scratch (reused across the 3 shift groups; DVE program order)
    NP3 = 3 * E
    PT = sb("PT", [P, 4 * NP3], AT)    # 4 packed products
    S1 = sb("S1", [P, 2 * NP3], AT)
    QT = sb("QT", [P, 4 * NP3], AT)    # [q1|q4|q2|q3]
    DT = sb("DT", [P, 2 * NP3], AT)
    GT = sb("GT", [P, 2 * NP3], BF16)
    avr = sb("avr", [P, NP3], AT); avi = sb("avi", [P, NP3], AT)
    chis = sb("chis", [P, NP3]); gw = sb("gw", [P, NP3], BF16)
    term = sb("term", [P, NP3])

    with (
        nc.semaphore("dma_sem") as dma_sem,
        nc.semaphore("act_sem") as act_sem,
        nc.semaphore("vec_sem") as vec_sem,
        nc.Block() as block,
    ):
        # ------------- SYNC: DMAs -------------
        dma_order = []   # name -> index (1-based)
        @block.sync
        def _(sync):
            def dma(k, r):
                sync.dma_start(
                    tin[k][:, _rs(r)],
                    ins[k][r].rearrange("(p e) -> p e", p=P, e=E),
                ).then_inc(dma_sem, 16)
                dma_order.append((k, r))
            for r in range(R):
                dma("alpha1", r); dma("alpha2", r)
            for r in range(R):
                dma("gamma1", r); dma("gamma2", r)
            for r in range(R):
                dma("beta1", r)
            for r in range(R):
                dma("beta2", r)
            for r in range(R):
                dma("m", r)
            sync.wait_ge(vec_sem, 12)
            sync.dma_start(out_ap.rearrange("(p e) -> p e", p=P, e=E),
                           acc[:]).then_inc(dma_sem, 16)
            sync.wait_ge(dma_sem, 29 * 16)

        def dma_count(k, upto_r=R):
            # DMAs needed so tin[k][:, :upto_r] slices are resident
            n = 0
            for i, (kk, rr) in enumerate(dma_order):
                if kk == k and rr == upto_r - 1:
                    n = i + 1
            return n * 16

        # ------------- SCALAR (ACT) -------------
        # 1 cb1, 2 sb1, 3 cb2, 4 sb2, 5 cb2-copy, 6 sb2-copy,
        # 7 snA, 8 snB, 9 csA, 10 csB, 11 msq, 12-15 wp1, 16-19 wp2,
        # 20 snC, 21 snD, 22 csC, 23 csD, 24 wreh, 25 wimh, 26 acc-init
        @block.scalar
        def _(scalar):
            scalar.wait_ge(vec_sem, 1)   # pi2 memset
            scalar.wait_ge(sem_b1, 64)
            scalar.activation(CB1[:, 0:W], tin["beta1"][:], ACTF.Sin, scale=0.5,
                              bias=pi2[:]).then_inc(act_sem, 1)        # 1 cb1
            scalar.activation(CB1[:, W:2*W], tin["beta1"][:], ACTF.Sin,
                              scale=0.5).then_inc(act_sem, 1)          # 2 sb1
            scalar.wait_ge(sem_b2, 64)
            scalar.activation(CB2a[:, 0:W], tin["beta2"][:], ACTF.Sin, scale=0.5,
                              bias=pi2[:]).then_inc(act_sem, 1)        # 3 cb2
            scalar.activation(CB2a[:, W:2*W], tin["beta2"][:], ACTF.Sin,
                              scale=0.5).then_inc(act_sem, 1)          # 4 sb2
            scalar.activation(CB2b[:, W:2*W], CB2a[:, 0:W],
                              ACTF.Copy).then_inc(act_sem, 1)          # 5 cb2 dup
            scalar.activation(CB2b[:, 0:W], CB2a[:, W:2*W],
                              ACTF.Copy).then_inc(act_sem, 1)          # 6 sb2 dup
            scalar.wait_ge(vec_sem, 5)   # all f tiles written
            for dst, n in ((SC1s[:, 0:W], "As"), (SC1s[:, W:2*W], "Bs"),
                           (SC1c[:, 0:W], "Ac"), (SC1c[:, W:2*W], "Bc")):
                scalar.activation(dst, fts[n][:], ACTF.Sin,
                                  scale=TWOPI).then_inc(act_sem, 1)    # 7..10
            scalar.wait_ge(sem_m, 64)
            scalar.activation(msq[:], tin["m"][:], ACTF.Square).then_inc(act_sem, 1)  # 11
            for r in range(R):
                scalar.activation(wp1[:, _rs(r)], msq[:, _rs(r)], ACTF.Copy,
                                  scale=-cR[r],
                                  bias=float(f32(cR[r]*m0sq[r] + k1[r]))
                                  ).then_inc(act_sem, 1)               # 12..15
            for r in range(R):
                scalar.activation(wp2[:, _rs(r)], msq[:, _rs(r)], ACTF.Copy,
                                  scale=-cI[r],
                                  bias=float(f32(cI[r]*m0sq[r] + k2[r]))
                                  ).then_inc(act_sem, 1)               # 16..19
            for dst, n in ((SC2s[:, 0:W], "Cs"), (SC2s[:, W:2*W], "Ds"),
                           (SC2c[:, 0:W], "Cc"), (SC2c[:, W:2*W], "Dc")):
                scalar.activation(dst, fts[n][:], ACTF.Sin,
                                  scale=TWOPI).then_inc(act_sem, 1)    # 20..23
            scalar.wait_ge(vec_sem, 6)   # wre
            scalar.activation(wreh[:], wre[:], ACTF.Copy).then_inc(act_sem, 1)  # 24
            scalar.wait_ge(vec_sem, 7)   # wim
            scalar.activation(wimh[:], wim[:], ACTF.Copy).then_inc(act_sem, 1)  # 25
            scalar.wait_ge(vec_sem, 8)   # dg
            scalar.activation(acc[:], dg[:], ACTF.Copy, scale=7.0).then_inc(act_sem, 1)  # 26

        # ------------- VECTOR (DVE) -------------
        @block.vector
        def _(vector):
            nc.vector.memset(pi2[:], HALFPI)
            # bases, per r as DMA lands
            for r in range(R):
                s = _rs(r)
                vector.wait_ge(dma_sem, dma_count("alpha2", r + 1))
                nc.vector.tensor_add(ub[:, s], tin["alpha1"][:, s], tin["alpha2"][:, s])
                nc.vector.tensor_sub(vb[:, s], tin["alpha1"][:, s], tin["alpha2"][:, s])
            for r in range(R):
                s = _rs(r)
                vector.wait_ge(dma_sem, dma_count("gamma2", r + 1))
                nc.vector.tensor_add(wb[:, s], tin["gamma1"][:, s], tin["gamma2"][:, s])
                nc.vector.tensor_sub(zb[:, s], tin["gamma1"][:, s], tin["gamma2"][:, s])
            # range-reduced fractional turns; inc vec_sem 1..8
            specs = [("As", ub, wb, RR_ADD, 0.0), ("Ac", ub, wb, RR_ADD, 0.25),
                     ("Bs", vb, zb, RR_SUB, 0.0), ("Bc", vb, zb, RR_SUB, 0.25),
                     ("Cs", ub, zb, RR_ADD, 0.0), ("Cc", ub, zb, RR_ADD, 0.25),
                     ("Ds", vb, wb, RR_SUB, 0.0), ("Dc", vb, wb, RR_SUB, 0.25)]
            for n, xa, xb, op, off in specs:
                nc.vector._custom_dve(op, out=fts[n][:], in0=xa[:], in1=xb[:],
                                      s0=INV4PI, s1=off, imm2=MAGIC
                                      ).then_inc(vec_sem, 1)
            # Wigner magnitudes (ACT 1..4)
            vector.wait_ge(act_sem, 4)
            nc.vector.tensor_mul(Ms[0][:], cb1[:], cb2[:])
            nc.vector.tensor_mul(Ms[1][:], sb1[:], sb2_[:])
            nc.vector.tensor_mul(Ms[2][:], cb1[:], sb2_[:])
            nc.vector.tensor_mul(Ms[3][:], sb1[:], cb2[:])
            # pq products (ACT 5..12)
            for i, (n, M) in enumerate([("As", 0), ("Ac", 0), ("Bs", 1), ("Bc", 1),
                                        ("Cs", 2), ("Cc", 2), ("Ds", 3), ("Dc", 3)]):
                vector.wait_ge(act_sem, 5 + i)
                nc.vector.tensor_mul(pqs[n][:], Ms[M][:], sct[n][:])
            nc.vector.tensor_sub(are[:], pqs["Ac"][:], pqs["Bc"][:])
            nc.vector.tensor_sub(aim[:], pqs["Bs"][:], pqs["As"][:])
            nc.vector.tensor_add(bre[:], pqs["Cc"][:], pqs["Dc"][:])
            nc.vector.tensor_add(bim[:], pqs["Cs"][:], pqs["Ds"][:])
            # Breit-Wigner (msq from ACT #13)
            vector.wait_ge(act_sem, 13)
            for r in range(R):
                nc.vector._custom_dve(DEN, out=den[:, _rs(r)], in0=msq[:, _rs(r)],
                                      s0=m0sq[r], s1=ysq[r])
            nc.vector.reciprocal_approx_fast(out=rcp[:], in_=den[:])
            vector.wait_ge(act_sem, 17)
            nc.vector.tensor_mul(wre[:], wp1[:], rcp[:]).then_inc(vec_sem, 1)   # 9
            vector.wait_ge(act_sem, 21)
            nc.vector.tensor_mul(wim[:], wp2[:], rcp[:]).then_inc(vec_sem, 1)   # 10
            nc.vector._custom_dve(CHI, out=dall[:], in0=wre[:], in1=wim[:],
                                  s0=1.0, s1=1.0, imm2=0.0)
            nc.vector.tensor_add(dh[:], dall[:, 0:2*E], dall[:, 2*E:4*E])
            nc.vector.tensor_add(dg[:], dh[:, 0:E], dh[:, E:2*E]).then_inc(vec_sem, 1)  # 11
            # pairs
            vector.wait_ge(act_sem, 25)   # wreh/wimh ready (and 24 soon for acc)
            first_acc = [True]
            ab4v = AB4.rearrange("p (c w) -> p c w", c=4, w=W)
            whv = WH.rearrange("p (c w) -> p c w", c=2, w=W)
            for sig in (1, 2, 3):
                n = (R - sig) * E
                L = slice(0, n)
                Rr = slice(sig * E, sig * E + n)
                # 4 Re-products in one packed op: PT = [aLaR|iLiR|bLbR|jLjR]
                ptv = PT.rearrange("p (c w) -> p c w", c=4, w=NP3)
                nc.vector.tensor_mul(ptv[:, :, :n], ab4v[:, :, L], ab4v[:, :, Rr])
                nc.vector.tensor_add(
                    S1.rearrange("p (c w) -> p c w", c=2, w=NP3)[:, :, :n],
                    ptv[:, 0:2, :n], ptv[:, 2:4, :n])
                nc.vector.tensor_add(avr[:, :n], S1[:, :n], S1[:, NP3:NP3+n])
                # Im-products into QT quarters ordered [q1|q4|q2|q3] so the
                # two differences are one chunked op
                qtv = QT.rearrange("p (c w) -> p c w", c=4, w=NP3)
                nc.vector.tensor_mul(qtv[:, 0, :n], are[:, L], aim[:, Rr])  # q1
                nc.vector.tensor_mul(qtv[:, 1, :n], bim[:, L], bre[:, Rr])  # q4
                nc.vector.tensor_mul(qtv[:, 2, :n], aim[:, L], are[:, Rr])  # q2
                nc.vector.tensor_mul(qtv[:, 3, :n], bre[:, L], bim[:, Rr])  # q3
                nc.vector.tensor_sub(
                    DT.rearrange("p (c w) -> p c w", c=2, w=NP3)[:, :, :n],
                    qtv[:, 0:2, :n], qtv[:, 2:4, :n])
                nc.vector.tensor_add(avi[:, :n], DT[:, :n], DT[:, NP3:NP3+n])
                nc.vector._custom_dve(CHI, out=chis[:, :n], in0=avr[:, :n],
                                      in1=avi[:, :n], s0=20.0, s1=4.0, imm2=-6.0)
                gtv = GT.rearrange("p (c w) -> p c w", c=2, w=NP3)
                nc.vector.tensor_mul(gtv[:, :, :n], whv[:, :, L], whv[:, :, Rr])
                nc.vector.tensor_add(gw[:, :n], GT[:, :n], GT[:, NP3:NP3+n])
                nc.vector.tensor_mul(term[:, :n], chis[:, :n], gw[:, :n])
                if first_acc[0]:
                    vector.wait_ge(act_sem, 26)   # acc initialised by ACT
                    first_acc[0] = False
                for blk in range(R - sig):
                    last = (sig == 3)
                    inst = nc.vector.tensor_add(acc[:], acc[:],
                                                term[:, blk*E:(blk+1)*E])
                    if last:
                        inst.then_inc(vec_sem, 1)   # 12 -> releases output DMA

    nc.compile()
    return nc


_CACHE = {}


def kernel(alpha1, beta1, gamma1, alpha2, beta2, gamma2, m, m0, g0,
           coef_r, coef_i, _want_trace=False):
    key = (np.asarray(m0, np.float32).tobytes(), np.asarray(g0, np.float32).tobytes(),
           np.asarray(coef_r, np.float32).tobytes(), np.asarray(coef_i, np.float32).tobytes())
    if key not in _CACHE:
        _CACHE[key] = build(np.asarray(m0, np.float32), np.asarray(g0, np.float32),
                            np.asarray(coef_r, np.float32), np.asarray(coef_i, np.float32))
    nc = _CACHE[key]
    full = {"alpha1": alpha1, "beta1": beta1, "gamma1": gamma1,
            "alpha2": alpha2, "beta2": beta2, "gamma2": gamma2, "m": m}
    in_maps = []
    for i in range(N_CORES):
        sl = slice(i * N_CORE, (i + 1) * N_CORE)
        in_maps.append({k: np.ascontiguousarray(np.asarray(v, np.float32)[:, sl])
                        for k, v in full.items()})
    res = run_bass_kernel_spmd(nc, in_maps, core_ids=list(range(N_CORES)),
                               trace=_want_trace)
    out = np.concatenate([res.results[i]["out"] for i in range(N_CORES)])
    if _want_trace:
        kernel._last_result = res
    return out.astype(np.float32)
